# revision 3
# baseline (speedup 1.0000x reference)
"""GCN (3-layer message passing) distributed over 8 TRN2 NeuronCores.

Sharding: nodes split evenly across 8 cores (rows). Weights replicated.
Per layer: local matmul h = x @ W + b (node-major out via x^T-stationary
matmuls), gather of the rows each peer needs into an AllToAll send
buffer, AllToAll exchange, then a local segment-sum implemented as
one-hot matmuls (edges tiled 128 at a time, PSUM-accumulated per
128-node destination window), with bias/relu fused in the epilogue.

The exchange is split into two window-chunks (dst windows [0,25) and
[25,50)) so the two AllToAlls pipeline against the send gathers and
aggregation: while chunk 0's collective runs, chunk 1's gathers are
generated; while chunk 1's collective runs, chunk 0 aggregates and the
next layer's matmul starts on completed windows.

Layer 3's inclusion linear Wi is folded into W3 (segment_sum commutes
with right-matmul), so the last exchange is only 16 (padded 128) wide.

Everything data-dependent (pair row counts, edge tiling) is computed
host-side in preprocess(); all 8 cores run one SPMD graph whose shapes
depend only on those computed constants.
"""
import sys

sys.path.insert(0, "/opt/trn_rl_repo")

import numpy as np
import ml_dtypes

import concourse.bass as bass
import concourse.bacc as bacc
import concourse.mybir as mybir
import concourse.tile as tile
from concourse.bass_utils import run_bass_kernel_spmd

NC = 8
NCH = 2
BF16 = mybir.dt.bfloat16
F32 = mybir.dt.float32
I16 = mybir.dt.int16

last_exec_time_ns = None
last_results = None


def _wrap16(idx, ncols):
    """[n] int -> [128, n/16] int16 wrapped (idx i at [i%16, i//16]) and
    replicated to 128 partitions."""
    a = np.asarray(idx, np.int16).reshape(ncols, 16).T  # [16, n/16]
    return np.tile(a, (8, 1))


def preprocess(features, W1, b1, W2, b2, W3, b3, Wi, bi, src, dst):
    """Host-side sharding/setup. Returns (cfg dict, in_maps list)."""
    N, K1t = features.shape  # 50000, 1433
    E = src.shape[0]
    assert N % NC == 0
    NLOC = N // NC
    MBLK = (NLOC + 127) // 128
    NPAD = MBLK * 128

    TW = [768, 512, 128]          # h-table widths (bf16, 256B-aligned)
    K = [1536, TW[0], TW[1]]      # matmul contraction dims (128-aligned)
    KB = [k // 128 for k in K]

    # window chunks: [0, wsplit) and [wsplit, NW)
    NW = MBLK
    wsplit = NW // 2
    wchunks = [(0, wsplit), (wsplit, NW)]

    # ---- weights (fold Wi into W3), padded, bf16 ----
    W3f = (W3.astype(np.float64) @ Wi.astype(np.float64)).astype(np.float32)
    b3f = (b3.astype(np.float64) @ Wi.astype(np.float64)).astype(np.float32)

    def pad2(a, r, c):
        out = np.zeros((r, c), np.float32)
        out[: a.shape[0], : a.shape[1]] = a
        return out

    w1 = pad2(W1, K[0], TW[0]).astype(ml_dtypes.bfloat16)
    w2 = pad2(W2, K[1], TW[1]).astype(ml_dtypes.bfloat16)
    w3 = pad2(W3f, K[2], TW[2]).astype(ml_dtypes.bfloat16)
    b1p = np.tile(pad2(b1[None, :], 1, TW[0]), (128, 1))
    b2p = np.tile(pad2(b2[None, :], 1, TW[1]), (128, 1))
    b3p = np.tile(pad2(b3f[None, :], 1, TW[2]), (128, 1))
    bip = np.tile(pad2(bi[None, :], 1, TW[2]), (128, 1))

    # ---- per-core transposed features [K[0], NPAD] bf16 ----
    featTs = []
    for c in range(NC):
        ft = np.zeros((K[0], NPAD), np.float32)
        ft[:K1t, :NLOC] = features[c * NLOC : (c + 1) * NLOC].T
        featTs.append(ft.astype(ml_dtypes.bfloat16))

    # ---- graph structure ----
    src = np.asarray(src, np.int64)
    dst = np.asarray(dst, np.int64)
    owner = src // NLOC
    dcore = dst // NLOC
    ldst_all = dst - dcore * NLOC
    win_all = ldst_all // 128

    # unique sources per (owner o -> dest d) pair, per window-chunk
    uniq = [[[None] * NC for _ in range(NC)] for _ in range(NCH)]
    Ps = []
    for ch, (w0, w1c) in enumerate(wchunks):
        inw = (win_all >= w0) & (win_all < w1c)
        for d in range(NC):
            maskd = inw & (dcore == d)
            for o in range(NC):
                m = maskd & (owner == o)
                uniq[ch][o][d] = np.unique(src[m])
        P = max(len(uniq[ch][o][d]) for o in range(NC) for d in range(NC))
        Ps.append(((P + 127) // 128) * 128)

    # Send-gather call plan per chunk: each dest pair's sorted row list is
    # split into calls of <=1024 positions. Rows within a call span a narrow
    # range, so the gather's table AP can be sliced to static [r0, r1) bounds
    # (min/max over cores) -- Tile's range deps then let early calls start
    # while the matmul is still writing later h rows.
    scalls = [[] for _ in range(NCH)]  # (sidx_off, a2a_pos, csz, r0, r1)
    sidx_parts = [[] for _ in range(NC)]   # per sender core
    scnts = [[] for _ in range(NC)]
    sidx_off = 0
    for ch in range(NCH):
        P = Ps[ch]
        csizes = []
        off = 0
        while off < P:
            csizes.append(min(1024, P - off))
            off += 1024
        ncalls = len(csizes)
        r0s = np.full(NC * ncalls, NPAD, np.int64)
        r1s = np.zeros(NC * ncalls, np.int64)
        for o in range(NC):
            for d in range(NC):
                u = uniq[ch][o][d] - o * NLOC
                off = 0
                for kk, csz in enumerate(csizes):
                    seg = u[off : off + csz]
                    ci = d * ncalls + kk
                    if len(seg):
                        r0s[ci] = min(r0s[ci], seg[0])
                        r1s[ci] = max(r1s[ci], seg[-1] + 1)
                    off += csz
        r0s = np.minimum(r0s, r1s)
        r1s = np.maximum(r1s, r0s + 1)
        # k-major order: calls needing only early h rows first
        order = [(kk, d) for kk in range(ncalls) for d in range(NC)]
        for kk, d in order:
            ci = d * ncalls + kk
            csz = csizes[kk]
            scalls[ch].append(
                (sidx_off, d * P + kk * 1024, csz, int(r0s[ci]), int(r1s[ci])))
            for o in range(NC):
                u = uniq[ch][o][d] - o * NLOC
                seg = u[kk * 1024 : kk * 1024 + csz] - r0s[ci]
                sidx_parts[o].append(np.concatenate(
                    [seg, np.full(csz - len(seg), -1, np.int64)]))
                scnts[o].append(len(seg))
            sidx_off += csz
    SIDX_N = sidx_off
    sidx = [_wrap16(np.concatenate(parts), SIDX_N // 16) for parts in sidx_parts]
    scnt = [np.asarray(c, np.int32).reshape(1, -1) for c in scnts]
    NSC = scnt[0].shape[1]

    # edge tiling per dest core, per chunk: tiles of 128 edges within 128-dst
    # windows; table positions index the chunk's recv table
    per_core = []
    for d in range(NC):
        m = dcore == d
        es, ed = src[m], dst[m]
        eo = es // NLOC
        ldst = ed - d * NLOC
        win = ldst // 128
        per_core.append((es, eo, win, ldst % 128))

    # common tiles-per-window across cores
    Tw = np.ones(NW, np.int64)
    for d in range(NC):
        _, _, win, _ = per_core[d]
        cnt = np.bincount(win, minlength=NW)
        Tw = np.maximum(Tw, (cnt + 127) // 128)
    tile_start = np.concatenate([[0], np.cumsum(Tw)])
    TT = int(tile_start[-1])
    T8 = ((TT + 7) // 8) * 8

    gidxs, dstrels, acnts = [], [], []
    for d in range(NC):
        es, eo, win, rel = per_core[d]
        # table position within this edge's chunk recv table
        chsel = np.where(win < wsplit, 0, 1)
        pos = np.empty(len(es), np.int64)
        for ch in range(NCH):
            for o in range(NC):
                mo = (eo == o) & (chsel == ch)
                pos[mo] = o * Ps[ch] + np.searchsorted(uniq[ch][o][d], es[mo])
        order = np.lexsort((pos, win))
        pos, win, rel = pos[order], win[order], rel[order]
        g = np.full(T8 * 128, -1, np.int64)
        r = np.full(T8 * 128, -1.0, np.float32)
        cnts = np.zeros(NW, np.int32)
        for w in range(NW):
            m = win == w
            n = int(m.sum())
            cnts[w] = n
            off = int(tile_start[w]) * 128
            g[off : off + n] = pos[m]
            r[off : off + n] = rel[m]
        gidxs.append(_wrap16(g, T8 * 8))
        dstrels.append(np.ascontiguousarray(r.reshape(T8, 128).T))
        acnts.append(cnts.reshape(1, NW))

    iota = np.tile(np.arange(128, dtype=np.float32)[None, :], (128, 1))

    cfg = dict(NLOC=NLOC, NPAD=NPAD, MBLK=MBLK, TW=TW, K=K, KB=KB, Ps=Ps,
               scalls=scalls, NSC=NSC, SIDX_N=SIDX_N,
               wchunks=wchunks, wsplit=wsplit,
               Tw=[int(x) for x in Tw],
               tile_start=[int(x) for x in tile_start],
               T8=T8, OUT_W=16)

    in_maps = []
    for c in range(NC):
        in_maps.append({
            "featT": featTs[c],
            "w1": w1, "w2": w2, "w3": w3,
            "b1": b1p, "b2": b2p, "b3": b3p, "bi": bip,
            "sidx": sidx[c], "gidx": gidxs[c], "drel": dstrels[c],
            "acnt": acnts[c], "scnt": scnt[c], "iota": iota,
        })
    return cfg, in_maps


def build(cfg, nq=4):
    NLOC, NPAD, MBLK = cfg["NLOC"], cfg["NPAD"], cfg["MBLK"]
    TW, K, KB, Ps = cfg["TW"], cfg["K"], cfg["KB"], cfg["Ps"]
    Tw, tile_start, T8 = cfg["Tw"], cfg["tile_start"], cfg["T8"]
    scalls, NSC, SIDX_N = cfg["scalls"], cfg["NSC"], cfg["SIDX_N"]
    wchunks = cfg["wchunks"]
    OUT_W = cfg["OUT_W"]
    NW = MBLK

    AGP_BUFS = 4
    nc = bacc.Bacc("TRN2", target_bir_lowering=False, debug=False,
                   num_devices=NC, num_swdge_queues=nq)

    featT = nc.declare_dram_parameter("featT", [K[0], NPAD], BF16, isOutput=False)
    wts = [nc.declare_dram_parameter(f"w{l+1}", [K[l], TW[l]], BF16, isOutput=False)
           for l in range(3)]
    bs = [nc.declare_dram_parameter(f"b{l+1}", [128, TW[l]], F32, isOutput=False)
          for l in range(3)]
    bi = nc.declare_dram_parameter("bi", [128, TW[2]], F32, isOutput=False)
    sidx = nc.declare_dram_parameter("sidx", [128, SIDX_N // 16], I16, isOutput=False)
    gidx = nc.declare_dram_parameter("gidx", [128, T8 * 8], I16, isOutput=False)
    drel = nc.declare_dram_parameter("drel", [128, T8], F32, isOutput=False)
    acnt = nc.declare_dram_parameter("acnt", [1, MBLK], mybir.dt.int32, isOutput=False)
    scnt = nc.declare_dram_parameter("scnt", [1, NSC], mybir.dt.int32, isOutput=False)
    iota = nc.declare_dram_parameter("iota", [128, 128], F32, isOutput=False)
    out = nc.declare_dram_parameter("out", [NLOC, OUT_W], F32, isOutput=True)

    hloc = [nc.dram_tensor(f"hloc{l}", [NPAD, TW[l]], BF16) for l in range(3)]
    a2ain = [[nc.dram_tensor(f"a2ain{l}_{ch}", [NC * Ps[ch], TW[l]], BF16)
              for ch in range(NCH)] for l in range(3)]
    recv = [[nc.dram_tensor(f"recv{l}_{ch}", [NC * Ps[ch], TW[l]], BF16)
             for ch in range(NCH)] for l in range(3)]
    xs = [None, nc.dram_tensor("x2", [NPAD, TW[0]], BF16),
          nc.dram_tensor("x3", [NPAD, TW[1]], BF16)]

    groups = [list(range(NC))]

    with tile.TileContext(nc) as tc:
        with (
            tc.tile_pool(name="wpool", bufs=1) as wpool,
            tc.tile_pool(name="bpool", bufs=1) as bpool,
            tc.tile_pool(name="ipool", bufs=1) as ipool,
            tc.tile_pool(name="xtp", bufs=2) as xtp,
            tc.tile_pool(name="mmpsum", bufs=2, space="PSUM") as mmpsum,
            tc.tile_pool(name="hbp", bufs=3) as hbp,
            tc.tile_pool(name="sgp", bufs=6) as sgp,
            tc.tile_pool(name="agp", bufs=AGP_BUFS) as agp,
            tc.tile_pool(name="ohp", bufs=6) as ohp,
            tc.tile_pool(name="apsum", bufs=2, space="PSUM") as apsum,
            tc.tile_pool(name="xop", bufs=3) as xop,
        ):
            # resident: indices, iota, dstrel
            sidx_t = ipool.tile([128, SIDX_N // 16], I16, tag="sidx")
            nc.sync.dma_start(sidx_t[:], sidx[:])
            gidx_t = ipool.tile([128, T8 * 8], I16, tag="gidx")
            nc.sync.dma_start(gidx_t[:], gidx[:])
            drel_t = ipool.tile([128, T8], F32, tag="drel")
            nc.sync.dma_start(drel_t[:], drel[:])
            iota_t = ipool.tile([128, 128], F32, tag="iota")
            nc.sync.dma_start(iota_t[:], iota[:])
            obuf = ipool.tile([128, NW, OUT_W], F32, tag="obuf")
            acnt_t = ipool.tile([1, NW], mybir.dt.int32, tag="acnt")
            nc.sync.dma_start(acnt_t[:], acnt[:])
            scnt_t = ipool.tile([1, NSC], mybir.dt.int32, tag="scnt")
            nc.sync.dma_start(scnt_t[:], scnt[:])
            TWMAX = max(Tw)
            # zero the gather slots once so rows skipped by short gathers
            # (num_idxs_reg < num_idxs) read as finite values
            for _ in range(AGP_BUFS):
                zt = agp.tile([128, TWMAX, max(TW)], BF16, tag="ag")
                nc.vector.memset(zt[:], 0.0)
            nreg = nc.gpsimd.alloc_register()

            for l in range(3):
              with nc.named_scope(f"L{l}"):
                  # ---- resident weights/bias for this layer ----
                  wt = wpool.tile([128, KB[l], TW[l]], BF16, tag="w")
                  nc.sync.dma_start(
                      wt[:], wts[l].rearrange("(kb p) w -> p kb w", p=128))
                  bt = bpool.tile([128, TW[l]], F32, tag="b")
                  nc.sync.dma_start(bt[:], bs[l][:])
                  if l == 2:
                      bit = bpool.tile([128, TW[2]], F32, tag="bi")
                      nc.sync.dma_start(bit[:], bi[:])

                  # ---- matmul: h = x @ W + b  (node-major PSUM out) ----
                  nslices = [(s, min(s + 512, TW[l])) for s in range(0, TW[l], 512)]
                  sc_mm = nc.enter_named_scope(f"mm{l}", False)[0]
                  NRW = 512
                  for nr in range(0, NPAD, NRW):
                      rw = min(NRW, NPAD - nr)
                      stripes = []
                      for kb in range(KB[l]):
                          st = xtp.tile([128, NRW], BF16, tag=f"xt{kb}")
                          if l == 0:
                              nc.sync.dma_start(
                                  st[:, :rw],
                                  featT[kb * 128 : (kb + 1) * 128, nr : nr + rw])
                          else:
                              nc.sync.dma_start_transpose(
                                  st[:, :rw],
                                  xs[l][nr : nr + rw, kb * 128 : (kb + 1) * 128])
                          stripes.append(st)
                      for m in range(rw // 128):
                          ps = mmpsum.tile([128, TW[l]], F32, tag="mmps")
                          for kb in range(KB[l]):
                              for (s0, s1) in nslices:
                                  nc.tensor.matmul(
                                      ps[:, s0:s1],
                                      stripes[kb][:, m * 128 : (m + 1) * 128],
                                      wt[:, kb, s0:s1],
                                      start=(kb == 0), stop=(kb == KB[l] - 1))
                          hb = hbp.tile([128, TW[l]], BF16, tag="hb")
                          nc.vector.tensor_tensor(
                              hb[:], ps[:], bt[:], op=mybir.AluOpType.add)
                          nc.sync.dma_start(
                              hloc[l][nr + m * 128 : nr + (m + 1) * 128, :], hb[:])

                  nc.leave_named_scope(f"mm{l}", sc_mm, False)
                  # ---- send gather + exchange, per window-chunk ----
                  for ch in range(NCH):
                      sc_sg = nc.enter_named_scope(f"sg{l}_{ch}", False)[0]
                      for ncall, (soff, pos0, csz, r0, r1) in enumerate(scalls[ch]):
                          ci = (0 if ch == 0 else len(scalls[0])) + ncall
                          g = sgp.tile([128, 8, TW[l]], BF16, tag="sg")
                          nb = csz // 128
                          nc.gpsimd.reg_load(nreg, scnt_t[0:1, ci : ci + 1])
                          nc.gpsimd.dma_gather(
                              g[:, :nb, :], hloc[l][r0:r1],
                              sidx_t[:, soff // 16 : (soff + csz) // 16],
                              csz, nreg, TW[l], queue_num=ci % nq)
                          nc.sync.dma_start(
                              a2ain[l][ch][pos0 : pos0 + csz, :]
                              .rearrange("(b p) w -> p b w", p=128), g[:, :nb, :])
                      nc.gpsimd.collective_compute(
                          "AllToAll", mybir.AluOpType.bypass,
                          replica_groups=groups,
                          ins=[a2ain[l][ch][:]], outs=[recv[l][ch][:]])
                      nc.leave_named_scope(f"sg{l}_{ch}", sc_sg, False)
                  # ---- aggregation: segment-sum via one-hot matmuls ----
                  sc_ag = nc.enter_named_scope(f"agg{l}", False)[0]
                  for ch in range(NCH):
                    for w in range(*wchunks[ch]):
                      ps = apsum.tile([128, TW[l]], F32, tag="aps")
                      t0 = tile_start[w]
                      # per-window gather; pad rows skipped via count reg
                      gt = agp.tile([128, TWMAX, TW[l]], BF16, tag="ag")
                      nc.gpsimd.reg_load(nreg, acnt_t[0:1, w : w + 1])
                      nc.gpsimd.dma_gather(
                          gt[:, : Tw[w], :], recv[l][ch][:],
                          gidx_t[:, t0 * 8 : (t0 + Tw[w]) * 8],
                          Tw[w] * 128, nreg, TW[l], queue_num=w % nq)
                      for tl in range(Tw[w]):
                          t = t0 + tl
                          oh = ohp.tile([128, 128], BF16, tag="oh")
                          nc.vector.tensor_scalar(
                              oh[:], iota_t[:], drel_t[:, t : t + 1], None,
                              mybir.AluOpType.is_equal)
                          rhs = gt[:, tl, :]
                          for (s0, s1) in nslices:
                              nc.tensor.matmul(
                                  ps[:, s0:s1], oh[:], rhs[:, s0:s1],
                                  start=(tl == 0), stop=(tl == Tw[w] - 1))
                      # ---- epilogue ----
                      if l < 2:
                          xb = xop.tile([128, TW[l]], BF16, tag="xo")
                          nc.vector.tensor_scalar_max(xb[:], ps[:], 0.0)
                          nc.sync.dma_start(
                              xs[l + 1][w * 128 : (w + 1) * 128, :], xb[:])
                      else:
                          nc.vector.tensor_tensor(
                              obuf[:, w, :], ps[:, :OUT_W], bit[:, :OUT_W],
                              op=mybir.AluOpType.add)
                          nc.vector.tensor_scalar_max(
                              obuf[:, w, :], obuf[:, w, :], 0.0)
                  if l == 2:
                      # one batched store for the full windows, then the tail
                      WFULL = NLOC // 128
                      nc.sync.dma_start(
                          out[: WFULL * 128, :]
                          .rearrange("(w p) c -> p w c", p=128),
                          obuf[:, :WFULL, :])
                      rows = NLOC - WFULL * 128
                      if rows > 0:
                          nc.sync.dma_start(
                              out[WFULL * 128 :, :], obuf[:rows, WFULL, :])
                  nc.leave_named_scope(f"agg{l}", sc_ag, False)
    nc.finalize()
    return nc


def kernel(**inputs):
    global last_exec_time_ns, last_results
    inputs = {k: np.asarray(v) for k, v in inputs.items()}
    cfg, in_maps = preprocess(**inputs)
    nc = build(cfg)
    res = None
    # trace=True needs the axon NTFF hook; fall back to untraced runs, and
    # retry once more on transient device errors (NRT_EXEC_UNIT_UNRECOVERABLE).
    for attempt, trace in enumerate([True, False, False]):
        try:
            res = run_bass_kernel_spmd(
                nc, in_maps, core_ids=list(range(NC)), trace=trace)
            break
        except Exception:
            if attempt == 2:
                raise
            import time
            time.sleep(15)
    last_exec_time_ns = res.exec_time_ns
    last_results = res
    return np.concatenate([res.results[c]["out"] for c in range(NC)], axis=0)


# revision 6
# speedup vs baseline: 1.1630x; 1.1630x over previous
"""GCN (3-layer message passing) distributed over 8 TRN2 NeuronCores.

Sharding: nodes split evenly across 8 cores (rows). Weights replicated.
Per layer: local matmul h = x @ W + b (node-major out via x^T-stationary
matmuls), then an AllGather of the local h table (two row-chunks, so the
first AllGather overlaps the second half of the matmul), then a local
segment-sum over incoming edges implemented as one-hot matmuls (edges
tiled 128 at a time, gathered per destination window from the replicated
h table, PSUM-accumulated per 128-node destination window), with
bias/relu fused in the epilogue.

AllGather replaces the send-gather + AllToAll of unique rows: it moves
more bytes but runs ~2.4x faster per byte on this fabric (KangaRing
1R2W vs Mesh AllToAll) and deletes the entire send-side gather
(descriptor generation on GpSimd was the top serial cost). The two row
chunks also keep gather table positions under the int16 index limit
(8 * 3200 = 25600 < 32767).

Layer 3's inclusion linear Wi is folded into W3 (segment_sum commutes
with right-matmul), so the last table is only 16 (padded 128) wide.

Everything data-dependent (edge tiling, counts) is computed host-side in
preprocess(); all 8 cores run one SPMD graph whose shapes depend only on
those computed constants.
"""
import sys

sys.path.insert(0, "/opt/trn_rl_repo")

import numpy as np
import ml_dtypes

import concourse.bass as bass
import concourse.bacc as bacc
import concourse.mybir as mybir
import concourse.tile as tile
from concourse.bass_utils import run_bass_kernel_spmd

NC = 8
NCH = 2
BF16 = mybir.dt.bfloat16
F32 = mybir.dt.float32
I16 = mybir.dt.int16

last_exec_time_ns = None
last_results = None


def _wrap16(idx, ncols):
    """[n] int -> [128, n/16] int16 wrapped (idx i at [i%16, i//16]) and
    replicated to 128 partitions."""
    a = np.asarray(idx, np.int16).reshape(ncols, 16).T  # [16, n/16]
    return np.tile(a, (8, 1))


def preprocess(features, W1, b1, W2, b2, W3, b3, Wi, bi, src, dst):
    """Host-side sharding/setup. Returns (cfg dict, in_maps list)."""
    N, K1t = features.shape  # 50000, 1433
    E = src.shape[0]
    assert N % NC == 0
    NLOC = N // NC
    MBLK = (NLOC + 127) // 128
    NPAD = MBLK * 128
    RH = NPAD // NCH  # rows per AllGather chunk

    TW = [768, 512, 128]          # h-table widths (bf16, 256B-aligned)
    K = [1536, TW[0], TW[1]]      # matmul contraction dims (128-aligned)
    KB = [k // 128 for k in K]

    # ---- weights (fold Wi into W3), padded, bf16 ----
    W3f = (W3.astype(np.float64) @ Wi.astype(np.float64)).astype(np.float32)
    b3f = (b3.astype(np.float64) @ Wi.astype(np.float64)).astype(np.float32)

    def pad2(a, r, c):
        out = np.zeros((r, c), np.float32)
        out[: a.shape[0], : a.shape[1]] = a
        return out

    w1 = pad2(W1, K[0], TW[0]).astype(ml_dtypes.bfloat16)
    w2 = pad2(W2, K[1], TW[1]).astype(ml_dtypes.bfloat16)
    w3 = pad2(W3f, K[2], TW[2]).astype(ml_dtypes.bfloat16)
    b1p = np.tile(pad2(b1[None, :], 1, TW[0]), (128, 1))
    b2p = np.tile(pad2(b2[None, :], 1, TW[1]), (128, 1))
    b3p = np.tile(pad2(b3f[None, :], 1, TW[2]), (128, 1))
    bip = np.tile(pad2(bi[None, :], 1, TW[2]), (128, 1))

    # ---- per-core transposed features [K[0], NPAD] bf16 ----
    featTs = []
    for c in range(NC):
        ft = np.zeros((K[0], NPAD), np.float32)
        ft[:K1t, :NLOC] = features[c * NLOC : (c + 1) * NLOC].T
        featTs.append(ft.astype(ml_dtypes.bfloat16))

    # ---- graph structure ----
    src = np.asarray(src, np.int64)
    dst = np.asarray(dst, np.int64)
    owner = src // NLOC
    dcore = dst // NLOC
    NW = MBLK

    # Edge tiling per dest core: within each 128-dst window, edges are
    # grouped by which AllGather chunk their source row lives in, then
    # tiled 128 at a time per (window, chunk) group. Gather positions
    # index the chunk's replicated table [NC * RH rows].
    lr_all = src - owner * NLOC          # local row in owner's hloc
    ch_all = (lr_all >= RH).astype(np.int64)
    pos_all = owner * RH + (lr_all - ch_all * RH)

    per_core = []
    for d in range(NC):
        m = dcore == d
        ldst = dst[m] - d * NLOC
        per_core.append((pos_all[m], ch_all[m], ldst // 128, ldst % 128))

    # common tiles-per-(window,chunk) across cores
    Twc = np.ones((NW, NCH), np.int64)
    for d in range(NC):
        _, ch, win, _ = per_core[d]
        for c in range(NCH):
            cnt = np.bincount(win[ch == c], minlength=NW)
            Twc[:, c] = np.maximum(Twc[:, c], (cnt + 127) // 128)
    # tile order: w-major, chunk inner
    tile_start = np.zeros((NW, NCH), np.int64)
    t = 0
    for w in range(NW):
        for c in range(NCH):
            tile_start[w, c] = t
            t += Twc[w, c]
    TT = t
    T8 = ((TT + 7) // 8) * 8

    gidxs, dstrels, acnts = [], [], []
    for d in range(NC):
        pos, ch, win, rel = per_core[d]
        order = np.lexsort((pos, ch, win))
        pos, ch, win, rel = pos[order], ch[order], win[order], rel[order]
        g = np.full(T8 * 128, -1, np.int64)
        r = np.full(T8 * 128, -1.0, np.float32)
        cnts = np.zeros((NW, NCH), np.int32)
        for w in range(NW):
            for c in range(NCH):
                m = (win == w) & (ch == c)
                n = int(m.sum())
                cnts[w, c] = n
                off = int(tile_start[w, c]) * 128
                g[off : off + n] = pos[m]
                r[off : off + n] = rel[m]
        gidxs.append(_wrap16(g, T8 * 8))
        # host-built one-hot tiles: ohm[p, t*128 + c] = (rel(t, slot p) == c)
        rel_pt = r.reshape(T8, 128).T  # [128 slots, T8]
        ohm = (rel_pt[:, :, None] == np.arange(128, dtype=np.float32)[None, None, :])
        dstrels.append(np.ascontiguousarray(
            ohm.reshape(128, T8 * 128).astype(ml_dtypes.bfloat16)))
        acnts.append(cnts.reshape(1, NW * NCH))

    cfg = dict(NLOC=NLOC, NPAD=NPAD, MBLK=MBLK, RH=RH, TW=TW, K=K, KB=KB,
               Twc=Twc.tolist(),
               tile_start=tile_start.tolist(),
               T8=T8, OUT_W=16)

    in_maps = []
    for c in range(NC):
        in_maps.append({
            "featT": featTs[c],
            "w1": w1, "w2": w2, "w3": w3,
            "b1": b1p, "b2": b2p, "b3": b3p, "bi": bip,
            "gidx": gidxs[c], "ohm": dstrels[c],
            "acnt": acnts[c],
        })
    return cfg, in_maps


def build(cfg, nq=4):
    NLOC, NPAD, MBLK, RH = cfg["NLOC"], cfg["NPAD"], cfg["MBLK"], cfg["RH"]
    TW, K, KB = cfg["TW"], cfg["K"], cfg["KB"]
    Twc, tile_start, T8 = cfg["Twc"], cfg["tile_start"], cfg["T8"]
    OUT_W = cfg["OUT_W"]
    NW = MBLK

    AGP_BUFS = 4
    nc = bacc.Bacc("TRN2", target_bir_lowering=False, debug=False,
                   num_devices=NC, num_swdge_queues=nq)

    featT = nc.declare_dram_parameter("featT", [K[0], NPAD], BF16, isOutput=False)
    wts = [nc.declare_dram_parameter(f"w{l+1}", [K[l], TW[l]], BF16, isOutput=False)
           for l in range(3)]
    bs = [nc.declare_dram_parameter(f"b{l+1}", [128, TW[l]], F32, isOutput=False)
          for l in range(3)]
    bi = nc.declare_dram_parameter("bi", [128, TW[2]], F32, isOutput=False)
    gidx = nc.declare_dram_parameter("gidx", [128, T8 * 8], I16, isOutput=False)
    ohm = nc.declare_dram_parameter("ohm", [128, T8 * 128], BF16, isOutput=False)
    acnt = nc.declare_dram_parameter("acnt", [1, NW * NCH], mybir.dt.int32,
                                     isOutput=False)
    out = nc.declare_dram_parameter("out", [NLOC, OUT_W], F32, isOutput=True)

    hloc = [nc.dram_tensor(f"hloc{l}", [NPAD, TW[l]], BF16) for l in range(3)]
    recv = [[nc.dram_tensor(f"recv{l}_{ch}", [NC * RH, TW[l]], BF16,
                            addr_space="Shared")
             for ch in range(NCH)] for l in range(3)]

    xs = [None, nc.dram_tensor("x2", [NPAD, TW[0]], BF16),
          nc.dram_tensor("x3", [NPAD, TW[1]], BF16)]

    groups = [list(range(NC))]

    with tile.TileContext(nc) as tc:
        with (
            tc.tile_pool(name="wpool", bufs=1) as wpool,
            tc.tile_pool(name="bpool", bufs=1) as bpool,
            tc.tile_pool(name="ipool", bufs=1) as ipool,
            tc.tile_pool(name="xtp", bufs=2) as xtp,
            tc.tile_pool(name="mmpsum", bufs=2, space="PSUM") as mmpsum,
            tc.tile_pool(name="hbp", bufs=3) as hbp,
            tc.tile_pool(name="agp", bufs=AGP_BUFS) as agp,
            tc.tile_pool(name="apsum", bufs=2, space="PSUM") as apsum,
            tc.tile_pool(name="xop", bufs=3) as xop,
        ):
            # resident: indices, iota, dstrel
            gidx_t = ipool.tile([128, T8 * 8], I16, tag="gidx")
            nc.sync.dma_start(gidx_t[:], gidx[:])
            ohm_t = ipool.tile([128, T8 * 128], BF16, tag="ohm")
            nc.sync.dma_start(ohm_t[:], ohm[:])
            obuf = ipool.tile([128, NW, OUT_W], F32, tag="obuf")
            acnt_t = ipool.tile([1, NW * NCH], mybir.dt.int32, tag="acnt")
            nc.sync.dma_start(acnt_t[:], acnt[:])
            TWMAX = max(max(tw) for tw in Twc)
            # zero the gather slots once so rows skipped by short gathers
            # (num_idxs_reg < num_idxs) read as finite values
            for _ in range(AGP_BUFS):
                zt = agp.tile([128, TWMAX, max(TW)], BF16, tag="ag")
                nc.vector.memset(zt[:], 0.0)
            nreg = nc.gpsimd.alloc_register()

            for l in range(3):
              with nc.named_scope(f"L{l}"):
                  # ---- resident weights/bias for this layer ----
                  wt = wpool.tile([128, KB[l], TW[l]], BF16, tag="w")
                  nc.sync.dma_start(
                      wt[:], wts[l].rearrange("(kb p) w -> p kb w", p=128))
                  bt = bpool.tile([128, TW[l]], F32, tag="b")
                  nc.sync.dma_start(bt[:], bs[l][:])
                  if l == 2:
                      bit = bpool.tile([128, TW[2]], F32, tag="bi")
                      nc.sync.dma_start(bit[:], bi[:])

                  # ---- matmul: h = x @ W + b  (node-major PSUM out) ----
                  nslices = [(s, min(s + 512, TW[l])) for s in range(0, TW[l], 512)]
                  sc_mm = nc.enter_named_scope(f"mm{l}", False)[0]
                  NRW = 512
                  for nr in range(0, NPAD, NRW):
                      rw = min(NRW, NPAD - nr)
                      stripes = []
                      for kb in range(KB[l]):
                          st = xtp.tile([128, NRW], BF16, tag=f"xt{kb}")
                          if l == 0:
                              nc.sync.dma_start(
                                  st[:, :rw],
                                  featT[kb * 128 : (kb + 1) * 128, nr : nr + rw])
                          else:
                              nc.sync.dma_start_transpose(
                                  st[:, :rw],
                                  xs[l][nr : nr + rw, kb * 128 : (kb + 1) * 128])
                          stripes.append(st)
                      for m in range(rw // 128):
                          ps = mmpsum.tile([128, TW[l]], F32, tag="mmps")
                          for kb in range(KB[l]):
                              for (s0, s1) in nslices:
                                  nc.tensor.matmul(
                                      ps[:, s0:s1],
                                      stripes[kb][:, m * 128 : (m + 1) * 128],
                                      wt[:, kb, s0:s1],
                                      start=(kb == 0), stop=(kb == KB[l] - 1))
                          hb = hbp.tile([128, TW[l]], BF16, tag="hb")
                          nc.vector.tensor_tensor(
                              hb[:], ps[:], bt[:], op=mybir.AluOpType.add)
                          nc.sync.dma_start(
                              hloc[l][nr + m * 128 : nr + (m + 1) * 128, :], hb[:])

                  nc.leave_named_scope(f"mm{l}", sc_mm, False)
                  # ---- exchange: AllGather of hloc, two row chunks ----
                  sc_sg = nc.enter_named_scope(f"ag{l}", False)[0]
                  for ch in range(NCH):
                      nc.gpsimd.collective_compute(
                          "AllGather", mybir.AluOpType.bypass,
                          replica_groups=groups,
                          ins=[hloc[l][ch * RH : (ch + 1) * RH, :]],
                          outs=[recv[l][ch][:]])
                  nc.leave_named_scope(f"ag{l}", sc_sg, False)
                  # ---- aggregation: segment-sum via one-hot matmuls ----
                  sc_ag = nc.enter_named_scope(f"agg{l}", False)[0]
                  for w in range(NW):
                      ps = apsum.tile([128, TW[l]], F32, tag="aps")
                      ntile_w = sum(Twc[w])
                      done = 0
                      for ch in range(NCH):
                          t0 = tile_start[w][ch]
                          tw = Twc[w][ch]
                          # per-(window,chunk) gather; pad rows skipped via
                          # count reg
                          gt = agp.tile([128, TWMAX, TW[l]], BF16, tag="ag")
                          nc.gpsimd.reg_load(
                              nreg, acnt_t[0:1, w * NCH + ch : w * NCH + ch + 1])
                          nc.gpsimd.dma_gather(
                              gt[:, :tw, :], recv[l][ch][:],
                              gidx_t[:, t0 * 8 : (t0 + tw) * 8],
                              tw * 128, nreg, TW[l],
                              queue_num=(w * NCH + ch) % nq)
                          for tl in range(tw):
                              t = t0 + tl
                              rhs = gt[:, tl, :]
                              for (s0, s1) in nslices:
                                  nc.tensor.matmul(
                                      ps[:, s0:s1],
                                      ohm_t[:, t * 128 : (t + 1) * 128],
                                      rhs[:, s0:s1],
                                      start=(done == 0),
                                      stop=(done == ntile_w - 1))
                              done += 1
                      # ---- epilogue ----
                      if l < 2:
                          xb = xop.tile([128, TW[l]], BF16, tag="xo")
                          nc.vector.tensor_scalar_max(xb[:], ps[:], 0.0)
                          nc.sync.dma_start(
                              xs[l + 1][w * 128 : (w + 1) * 128, :], xb[:])
                      else:
                          nc.vector.tensor_tensor(
                              obuf[:, w, :], ps[:, :OUT_W], bit[:, :OUT_W],
                              op=mybir.AluOpType.add)
                          nc.vector.tensor_scalar_max(
                              obuf[:, w, :], obuf[:, w, :], 0.0)
                  if l == 2:
                      # one batched store for the full windows, then the tail
                      WFULL = NLOC // 128
                      nc.sync.dma_start(
                          out[: WFULL * 128, :]
                          .rearrange("(w p) c -> p w c", p=128),
                          obuf[:, :WFULL, :])
                      rows = NLOC - WFULL * 128
                      if rows > 0:
                          nc.sync.dma_start(
                              out[WFULL * 128 :, :], obuf[:rows, WFULL, :])
                  nc.leave_named_scope(f"agg{l}", sc_ag, False)
    nc.finalize()
    return nc


def kernel(**inputs):
    global last_exec_time_ns, last_results
    inputs = {k: np.asarray(v) for k, v in inputs.items()}
    cfg, in_maps = preprocess(**inputs)
    nc = build(cfg)
    res = None
    # trace=True needs the axon NTFF hook; fall back to untraced runs, and
    # retry once more on transient device errors (NRT_EXEC_UNIT_UNRECOVERABLE).
    for attempt, trace in enumerate([True, False, False]):
        try:
            res = run_bass_kernel_spmd(
                nc, in_maps, core_ids=list(range(NC)), trace=trace)
            break
        except Exception:
            if attempt == 2:
                raise
            import time
            time.sleep(15)
    last_exec_time_ns = res.exec_time_ns
    last_results = res
    return np.concatenate([res.results[c]["out"] for c in range(NC)], axis=0)
